# revision 21
# baseline (speedup 1.0000x reference)
"""Trainium2 Bass kernel for nn_PairwisePredictionHead.

Math (reference):
  xd = x @ W_down.T + b_down             # [L, 128]
  q, k = xd[:, :64], xd[:, 64:]
  h[i,j,:] = W1p @ (q_j*k_i) + W1d @ (q_j - k_i) + b1    # [L, L, 128]
  g = gelu_exact(h)
  out = W2 @ LN(g) + b2                   # [L, L, 64]

Sharding: row-shard i across 8 cores (96 rows each). Each core gets the full
q-side (all 768 j) plus its own 96 k-rows; cores are independent (no
collectives), outputs concatenated on host.

Device (per core, per pair-grid row i) — all matmuls bf16, j rides the free
dim so every stage-2 stationary is static (no per-chunk weight reloads):
  - lhsT_i = [[W1pT * k_i[:,None]] ; W1dT]  (top half rebuilt per i on DVE,
    two i's per op, software-pipelined one pair ahead of the matmuls)
  - p1[h, j]  = lhsT_i.T @ [q.T; q.T]          (PE, N=512+256, psum f32)
  - g  = Gelu(p1 + (b1 - W1d@k_i))             (ACT, psum -> sbuf bf16)
  - g2 = g*g                                   (DVE 2x, one op per pair)
  - o2[0:96, j] = [W2z*ln_g - rowmean | ones | 0pad].T @ g  (PE, static stat.;
    row 64 = column sums Sg since col 64 of the stationary is ones)
  - o2[96, j]   = ones.T @ g2   (PE col-tile (0,96): Sg2 row, own psum group)
  - psum -> sbuf bf16 copy split across DVE (cols 0:512) + ACT (512:768);
    rows i paired in one sbuf tile so output DMA fires once per pair, split
    across the SP and gpsimd DMA queues (DMA cannot read PSUM on trn2).
Host: downproject x -> q,k (0.2 of 3.9 GFLOP, f64) and the LN epilogue:
mu = Sg/128, var = Sg2/128 - mu^2, r = rsqrt(var+eps),
out[i,j,n] = po[i,n,j]*r[i,j] + c[n],  c = W2@ln_b + b2 (plus the [n,j] ->
[j,n] transpose during unshard). Zero-mean rows of W2z absorb LN's mean
subtraction (w.(g-mu) == (w-mean w).g), so no mean correction is needed on
device. gelu/LN-stat rsqrt never meet on device, which avoids the ACT
table-swap (gelu and rsqrt live in different activation table sets).
"""

import os

import numpy as np
import ml_dtypes

import concourse.mybir as mybir
import concourse.tile as tile
from concourse import bacc
from concourse.bass_utils import run_bass_kernel_spmd

F32 = mybir.dt.float32
BF16 = mybir.dt.bfloat16
AF = mybir.ActivationFunctionType
ALU = mybir.AluOpType

B, L, D = 1, 768, 1024
DP, H, NB = 128, 128, 64
NCORES = 8
ROWS = L // NCORES  # 96 pair-grid rows per core
P = 128
EPS = 1e-5


def _build(nc):
    qqd = nc.dram_tensor("qqd", [P, L], BF16, kind="ExternalInput")
    kTd = nc.dram_tensor("kTd", [64, ROWS], F32, kind="ExternalInput")
    b1cd = nc.dram_tensor("b1cd", [P, ROWS], F32, kind="ExternalInput")
    W1pT = nc.dram_tensor("W1pT", [64, P], F32, kind="ExternalInput")
    W1dT = nc.dram_tensor("W1dT", [64, P], BF16, kind="ExternalInput")
    W2zTe = nc.dram_tensor("W2zTe", [P, 96], BF16, kind="ExternalInput")
    po_out = nc.dram_tensor("po_out", [ROWS, 65, L], BF16, kind="ExternalOutput")
    sg2_out = nc.dram_tensor("sg2_out", [ROWS, L], BF16, kind="ExternalOutput")

    with tile.TileContext(nc) as tc:
        with tc.tile_pool(name="const", bufs=1) as const, \
             tc.tile_pool(name="work", bufs=5) as work, \
             tc.tile_pool(name="pp1", bufs=2, space="PSUM") as pp1, \
             tc.tile_pool(name="ppo", bufs=2, space="PSUM") as ppo, \
             tc.tile_pool(name="ppb", bufs=2, space="PSUM") as ppb:

            # ---- constants into SBUF (downproj done on host) ----
            # load order: first MM1's deps first (kT+W1pT for the lt build,
            # lhsT slots 0-1, qq), bulk/const tails later
            kT_sb = const.tile([64, ROWS], F32)
            nc.sync.dma_start(out=kT_sb, in_=kTd[:])
            W1pT_sb = const.tile([64, P], F32)
            nc.sync.dma_start(out=W1pT_sb, in_=W1pT[:])
            lhsT4 = const.tile([P, 8, P], BF16)
            for t in range(2):
                nc.sync.dma_start(out=lhsT4[64:128, t, :], in_=W1dT[:])
            qq = const.tile([P, L], BF16)
            nc.gpsimd.dma_start(out=qq[:, 0:384], in_=qqd[:, 0:384])
            nc.gpsimd.dma_start(out=qq[:, 384:768], in_=qqd[:, 384:768])
            W2zTe_sb = const.tile([P, 96], BF16)
            nc.scalar.dma_start(out=W2zTe_sb, in_=W2zTe[:])
            ones_sb = const.tile([P, 1], BF16)
            nc.vector.memset(ones_sb, 1.0)
            b1c = const.tile([P, ROWS], F32)
            nc.scalar.dma_start(out=b1c, in_=b1cd[:])
            for t in range(2, 8):
                eng = (nc.sync, nc.gpsimd, nc.scalar)[t % 3]
                eng.dma_start(out=lhsT4[64:128, t, :], in_=W1dT[:])
            W1pT_bc = W1pT_sb[:, None, :].broadcast_to([64, 2, P])

            # ---- main loop (paired i's; MM1s hoisted so the in-order PE
            # queue has both p1 matmuls runnable before parking on gelu) ----
            def build_lt(ip):
                i0 = 2 * ip
                s = (ip % 4) * 2  # lhsT4 slots s, s+1
                kcol = kT_sb[:, i0:i0 + 2, None].broadcast_to([64, 2, P])
                nc.vector.tensor_tensor(lhsT4[0:64, s:s + 2, :], W1pT_bc,
                                        kcol, ALU.mult)

            build_lt(0)
            for ip in range(ROWS // 2):
                i0 = 2 * ip
                s = (ip % 4) * 2
                if ip + 1 < ROWS // 2:
                    build_lt(ip + 1)

                gp = work.tile([P, 2, L], BF16, tag="g", name="g")
                g2p = work.tile([P, 2, L], BF16, tag="g2", name="g2")
                cp = work.tile([P, 2, L], BF16, tag="cp", name="cp")
                p1t = [None, None]
                for t in range(2):
                    p1 = pp1.tile([P, L], F32, tag="p1", name="p1")
                    p1t[t] = p1
                    nc.tensor.matmul(p1[:, 0:512], lhsT4[:, s + t, :],
                                     qq[:, 0:512], start=True, stop=True)
                    nc.tensor.matmul(p1[:, 512:768], lhsT4[:, s + t, :],
                                     qq[:, 512:768], start=True, stop=True)
                for t in range(2):
                    ii = i0 + t
                    nc.scalar.activation(gp[:, t, :], p1t[t], AF.Gelu,
                                         bias=b1c[:, ii:ii + 1])
                    if t == 1:
                        nc.vector.tensor_mul(g2p, gp, gp)
                o2t = [None, None]
                for t in range(2):
                    o2a = ppo.tile([P, 512], F32, tag="poa", name="o2a")
                    o2b = ppb.tile([P, 256], F32, tag="pob", name="o2b")
                    o2t[t] = (o2a, o2b)
                    nc.tensor.matmul(o2a[0:96, :], W2zTe_sb, gp[:, t, 0:512],
                                     start=True, stop=True)
                    nc.tensor.matmul(o2b[0:96, :], W2zTe_sb,
                                     gp[:, t, 512:768], start=True, stop=True)
                for t in range(2):
                    o2a, o2b = o2t[t]
                    nc.tensor.matmul(o2a[96:97, :], ones_sb,
                                     g2p[:, t, 0:512], start=True, stop=True,
                                     tile_position=(0, 96))
                    nc.tensor.matmul(o2b[96:97, :], ones_sb,
                                     g2p[:, t, 512:768], start=True,
                                     stop=True, tile_position=(0, 96))
                for t in range(2):
                    o2a, o2b = o2t[t]
                    nc.vector.tensor_copy(cp[0:97, t, 0:512], o2a[0:97, :])
                    nc.scalar.activation(cp[0:97, t, 512:768],
                                         o2b[0:97, :], AF.Identity)
                nc.sync.dma_start(
                    out=po_out[i0:i0 + 2, 0:33].rearrange("t p l -> p t l"),
                    in_=cp[0:33, :, :])
                nc.gpsimd.dma_start(
                    out=po_out[i0:i0 + 2, 33:65].rearrange("t p l -> p t l"),
                    in_=cp[33:65, :, :])
                nc.gpsimd.dma_start(out=sg2_out[i0:i0 + 2, :],
                                    in_=cp[96:97, :, :])

def host_prep(x, W_down, b_down, W1, b1, ln_g, ln_b, W2, b2):
    f32 = np.float32
    f64 = np.float64
    bf16 = ml_dtypes.bfloat16
    xd = x[0].astype(f64) @ W_down.astype(f64).T + b_down.astype(f64)  # [L,128]
    q = xd[:, :64]
    k = xd[:, 64:]
    qq = np.concatenate([q.T, q.T], axis=0)  # [128, L]
    common = {
        "qqd": np.ascontiguousarray(qq.astype(bf16)),
        "W1pT": np.ascontiguousarray(W1[:, :64].T.astype(f32)),
        "W1dT": np.ascontiguousarray(W1[:, 64:].T.astype(bf16)),
    }
    W2g = W2.astype(f64) * ln_g.astype(f64)[None, :]
    W2z = W2g - W2g.mean(axis=1, keepdims=True)
    W2zTe = np.concatenate([W2z.T, np.ones((P, 1)), np.zeros((P, 31))],
                           axis=1)  # [128, 96]
    common["W2zTe"] = np.ascontiguousarray(W2zTe.astype(bf16))
    cvec = (W2.astype(f64) @ ln_b.astype(f64) + b2.astype(f64)).astype(f32)
    W1d = W1[:, 64:].astype(f64)
    b1c_all = b1.astype(f64)[:, None] - W1d @ k.T  # [128, L]
    return common, k, b1c_all, cvec


def kernel(x, W_down, b_down, W1, b1, ln_g, ln_b, W2, b2):
    x = np.asarray(x)
    common, kfull, b1c_all, cvec = host_prep(
        x, np.asarray(W_down), np.asarray(b_down), np.asarray(W1),
        np.asarray(b1), np.asarray(ln_g), np.asarray(ln_b), np.asarray(W2),
        np.asarray(b2))

    nc = bacc.Bacc("TRN2")
    _build(nc)
    nc.finalize()

    in_maps = []
    for core in range(NCORES):
        m = dict(common)
        i0 = core * ROWS
        m["kTd"] = np.ascontiguousarray(
            kfull[i0:i0 + ROWS, :].T.astype(np.float32))
        m["b1cd"] = np.ascontiguousarray(
            b1c_all[:, i0:i0 + ROWS].astype(np.float32))
        in_maps.append(m)

    trace = os.environ.get("KERNEL_TRACE", "0") == "1"
    res = run_bass_kernel_spmd(nc, in_maps, core_ids=list(range(NCORES)),
                               trace=trace)
    if trace and res.exec_time_ns is not None:
        print(f"HW exec time: {res.exec_time_ns} ns")

    # Host epilogue: r = rsqrt(var + eps); out = po^T * r + c
    outs = []
    for core in range(NCORES):
        po = np.asarray(res.results[core]["po_out"]).astype(np.float32)
        sg2 = np.asarray(res.results[core]["sg2_out"]).astype(np.float32)
        mu = po[:, 64, :] * np.float32(1.0 / H)
        var = np.maximum(sg2 * np.float32(1.0 / H) - mu * mu, np.float32(0))
        r = 1.0 / np.sqrt(var + np.float32(EPS))          # [ROWS, L]
        o = po[:, :64, :] * r[:, None, :]                 # [ROWS, 64, L]
        outs.append(np.ascontiguousarray(o.transpose(0, 2, 1)) + cvec)
    full = np.concatenate(outs, axis=0)  # [768, 768, 64]
    return full[None].astype(np.float32)



# revision 22
# speedup vs baseline: 1.0342x; 1.0342x over previous
"""Trainium2 Bass kernel for nn_PairwisePredictionHead.

Math (reference):
  xd = x @ W_down.T + b_down             # [L, 128]
  q, k = xd[:, :64], xd[:, 64:]
  h[i,j,:] = W1p @ (q_j*k_i) + W1d @ (q_j - k_i) + b1    # [L, L, 128]
  g = gelu_exact(h)
  out = W2 @ LN(g) + b2                   # [L, L, 64]

Sharding: row-shard i across 8 cores (96 rows each). Each core gets the full
q-side (all 768 j) plus its own 96 k-rows; cores are independent (no
collectives), outputs concatenated on host.

Device (per core, per pair-grid row i) — all matmuls bf16, j rides the free
dim so every stage-2 stationary is static (no per-chunk weight reloads):
  - lhsT_i = [[W1pT * k_i[:,None]] ; W1dT]  (top half rebuilt per i on DVE,
    two i's per op, software-pipelined one pair ahead of the matmuls)
  - p1[h, j]  = lhsT_i.T @ [q.T; q.T]          (PE, N=512+256, psum f32)
  - g  = Gelu(p1 + (b1 - W1d@k_i))             (ACT, psum -> sbuf bf16)
  - g2 = g*g                                   (DVE 2x, one op per pair)
  - o2[0:96, j] = [W2z*ln_g - rowmean | ones | 0pad].T @ g  (PE, static stat.;
    row 64 = column sums Sg since col 64 of the stationary is ones)
  - o2[96, j]   = ones.T @ g2   (PE col-tile (0,96): Sg2 row, own psum group)
  - psum -> sbuf bf16 copy split across DVE (cols 0:512) + ACT (512:768);
    rows i paired in one sbuf tile so output DMA fires once per pair, split
    across the SP and gpsimd DMA queues (DMA cannot read PSUM on trn2).
Host: downproject x -> q,k (0.2 of 3.9 GFLOP, f64) and the LN epilogue:
mu = Sg/128, var = Sg2/128 - mu^2, r = rsqrt(var+eps),
out[i,j,n] = po[i,n,j]*r[i,j] + c[n],  c = W2@ln_b + b2 (plus the [n,j] ->
[j,n] transpose during unshard). Zero-mean rows of W2z absorb LN's mean
subtraction (w.(g-mu) == (w-mean w).g), so no mean correction is needed on
device. gelu/LN-stat rsqrt never meet on device, which avoids the ACT
table-swap (gelu and rsqrt live in different activation table sets).
"""

import os

import numpy as np
import ml_dtypes

import concourse.mybir as mybir
import concourse.tile as tile
from concourse import bacc
from concourse.bass_utils import run_bass_kernel_spmd

F32 = mybir.dt.float32
BF16 = mybir.dt.bfloat16
AF = mybir.ActivationFunctionType
ALU = mybir.AluOpType

B, L, D = 1, 768, 1024
DP, H, NB = 128, 128, 64
NCORES = 8
ROWS = L // NCORES  # 96 pair-grid rows per core
P = 128
EPS = 1e-5


def _build(nc):
    qqd = nc.dram_tensor("qqd", [P, L], BF16, kind="ExternalInput")
    kTd = nc.dram_tensor("kTd", [64, ROWS], F32, kind="ExternalInput")
    b1cd = nc.dram_tensor("b1cd", [P, ROWS], F32, kind="ExternalInput")
    W1pT = nc.dram_tensor("W1pT", [64, P], F32, kind="ExternalInput")
    W1dT = nc.dram_tensor("W1dT", [64, P], BF16, kind="ExternalInput")
    W2zTe = nc.dram_tensor("W2zTe", [P, 96], BF16, kind="ExternalInput")
    po_out = nc.dram_tensor("po_out", [ROWS, 65, L], BF16, kind="ExternalOutput")
    sg2_out = nc.dram_tensor("sg2_out", [ROWS, L], BF16, kind="ExternalOutput")

    with tile.TileContext(nc) as tc:
        with tc.tile_pool(name="const", bufs=1) as const, \
             tc.tile_pool(name="work", bufs=5) as work, \
             tc.tile_pool(name="pp1", bufs=2, space="PSUM") as pp1, \
             tc.tile_pool(name="ppo", bufs=2, space="PSUM") as ppo, \
             tc.tile_pool(name="ppb", bufs=2, space="PSUM") as ppb:

            # ---- constants into SBUF (downproj done on host) ----
            # load order: first MM1's deps first (kT+W1pT for the lt build,
            # lhsT slots 0-1, qq), bulk/const tails later
            kT_sb = const.tile([64, ROWS], F32)
            nc.sync.dma_start(out=kT_sb, in_=kTd[:])
            W1pT_sb = const.tile([64, P], F32)
            nc.sync.dma_start(out=W1pT_sb, in_=W1pT[:])
            lhsT4 = const.tile([P, 8, P], BF16)
            for t in range(2):
                nc.sync.dma_start(out=lhsT4[64:128, t, :], in_=W1dT[:])
            qq = const.tile([P, L], BF16)
            nc.gpsimd.dma_start(out=qq[:, 0:384], in_=qqd[:, 0:384])
            nc.gpsimd.dma_start(out=qq[:, 384:768], in_=qqd[:, 384:768])
            W2zTe_sb = const.tile([P, 96], BF16)
            nc.scalar.dma_start(out=W2zTe_sb, in_=W2zTe[:])
            ones_sb = const.tile([P, 1], BF16)
            nc.vector.memset(ones_sb, 1.0)
            b1c = const.tile([P, ROWS], F32)
            nc.scalar.dma_start(out=b1c, in_=b1cd[:])
            for t in range(2, 8):
                eng = (nc.sync, nc.gpsimd, nc.scalar)[t % 3]
                eng.dma_start(out=lhsT4[64:128, t, :], in_=W1dT[:])
            W1pT_bc = W1pT_sb[:, None, :].broadcast_to([64, 2, P])

            # ---- main loop (paired i's; MM1s hoisted so the in-order PE
            # queue has both p1 matmuls runnable before parking on gelu) ----
            def build_lt(ip):
                i0 = 2 * ip
                s = (ip % 4) * 2  # lhsT4 slots s, s+1
                kcol = kT_sb[:, i0:i0 + 2, None].broadcast_to([64, 2, P])
                nc.vector.tensor_tensor(lhsT4[0:64, s:s + 2, :], W1pT_bc,
                                        kcol, ALU.mult)

            build_lt(0)
            for ip in range(ROWS // 2):
                i0 = 2 * ip
                s = (ip % 4) * 2
                if ip + 1 < ROWS // 2:
                    build_lt(ip + 1)

                gp = work.tile([P, 2, L], BF16, tag="g", name="g")
                g2p = work.tile([P, 2, L], BF16, tag="g2", name="g2")
                cp = work.tile([P, 2, L], BF16, tag="cp", name="cp")
                p1t = [None, None]
                for t in range(2):
                    p1 = pp1.tile([P, L], F32, tag="p1", name="p1")
                    p1t[t] = p1
                    nc.tensor.matmul(p1[:, 0:512], lhsT4[:, s + t, :],
                                     qq[:, 0:512], start=True, stop=True)
                    nc.tensor.matmul(p1[:, 512:768], lhsT4[:, s + t, :],
                                     qq[:, 512:768], start=True, stop=True)
                for t in range(2):
                    ii = i0 + t
                    nc.scalar.activation(gp[:, t, :], p1t[t], AF.Gelu,
                                         bias=b1c[:, ii:ii + 1])
                    if t == 1:
                        nc.vector.tensor_mul(g2p, gp, gp)
                o2t = [None, None]
                for t in range(2):
                    o2a = ppo.tile([P, 512], F32, tag="poa", name="o2a")
                    o2b = ppb.tile([P, 256], F32, tag="pob", name="o2b")
                    o2t[t] = (o2a, o2b)
                    nc.tensor.matmul(o2a[0:96, :], W2zTe_sb, gp[:, t, 0:512],
                                     start=True, stop=True)
                    nc.tensor.matmul(o2b[0:96, :], W2zTe_sb,
                                     gp[:, t, 512:768], start=True, stop=True)
                for t in range(2):
                    o2a, o2b = o2t[t]
                    nc.tensor.matmul(o2a[96:97, :], ones_sb,
                                     g2p[:, t, 0:512], start=True, stop=True,
                                     tile_position=(0, 96))
                    nc.tensor.matmul(o2b[96:97, :], ones_sb,
                                     g2p[:, t, 512:768], start=True,
                                     stop=True, tile_position=(0, 96))
                for t in range(2):
                    o2a, o2b = o2t[t]
                    nc.vector.tensor_copy(cp[0:97, t, 0:512], o2a[0:97, :])
                    nc.scalar.activation(cp[0:97, t, 512:768],
                                         o2b[0:97, :], AF.Identity)
                nc.sync.dma_start(
                    out=po_out[i0:i0 + 2, 0:33].rearrange("t p l -> p t l"),
                    in_=cp[0:33, :, :])
                nc.gpsimd.dma_start(
                    out=po_out[i0:i0 + 2, 33:65].rearrange("t p l -> p t l"),
                    in_=cp[33:65, :, :])
                nc.sync.dma_start(out=sg2_out[i0:i0 + 2, :],
                                  in_=cp[96:97, :, :])

def host_prep(x, W_down, b_down, W1, b1, ln_g, ln_b, W2, b2):
    f32 = np.float32
    f64 = np.float64
    bf16 = ml_dtypes.bfloat16
    xd = x[0].astype(f64) @ W_down.astype(f64).T + b_down.astype(f64)  # [L,128]
    q = xd[:, :64]
    k = xd[:, 64:]
    qq = np.concatenate([q.T, q.T], axis=0)  # [128, L]
    common = {
        "qqd": np.ascontiguousarray(qq.astype(bf16)),
        "W1pT": np.ascontiguousarray(W1[:, :64].T.astype(f32)),
        "W1dT": np.ascontiguousarray(W1[:, 64:].T.astype(bf16)),
    }
    W2g = W2.astype(f64) * ln_g.astype(f64)[None, :]
    W2z = W2g - W2g.mean(axis=1, keepdims=True)
    W2zTe = np.concatenate([W2z.T, np.ones((P, 1)), np.zeros((P, 31))],
                           axis=1)  # [128, 96]
    common["W2zTe"] = np.ascontiguousarray(W2zTe.astype(bf16))
    cvec = (W2.astype(f64) @ ln_b.astype(f64) + b2.astype(f64)).astype(f32)
    W1d = W1[:, 64:].astype(f64)
    b1c_all = b1.astype(f64)[:, None] - W1d @ k.T  # [128, L]
    return common, k, b1c_all, cvec


def kernel(x, W_down, b_down, W1, b1, ln_g, ln_b, W2, b2):
    x = np.asarray(x)
    common, kfull, b1c_all, cvec = host_prep(
        x, np.asarray(W_down), np.asarray(b_down), np.asarray(W1),
        np.asarray(b1), np.asarray(ln_g), np.asarray(ln_b), np.asarray(W2),
        np.asarray(b2))

    nc = bacc.Bacc("TRN2")
    _build(nc)
    nc.finalize()

    in_maps = []
    for core in range(NCORES):
        m = dict(common)
        i0 = core * ROWS
        m["kTd"] = np.ascontiguousarray(
            kfull[i0:i0 + ROWS, :].T.astype(np.float32))
        m["b1cd"] = np.ascontiguousarray(
            b1c_all[:, i0:i0 + ROWS].astype(np.float32))
        in_maps.append(m)

    trace = os.environ.get("KERNEL_TRACE", "0") == "1"
    res = run_bass_kernel_spmd(nc, in_maps, core_ids=list(range(NCORES)),
                               trace=trace)
    if trace and res.exec_time_ns is not None:
        print(f"HW exec time: {res.exec_time_ns} ns")

    # Host epilogue: r = rsqrt(var + eps); out = po^T * r + c
    outs = []
    for core in range(NCORES):
        po = np.asarray(res.results[core]["po_out"]).astype(np.float32)
        sg2 = np.asarray(res.results[core]["sg2_out"]).astype(np.float32)
        mu = po[:, 64, :] * np.float32(1.0 / H)
        var = np.maximum(sg2 * np.float32(1.0 / H) - mu * mu, np.float32(0))
        r = 1.0 / np.sqrt(var + np.float32(EPS))          # [ROWS, L]
        o = po[:, :64, :] * r[:, None, :]                 # [ROWS, 64, L]
        outs.append(np.ascontiguousarray(o.transpose(0, 2, 1)) + cvec)
    full = np.concatenate(outs, axis=0)  # [768, 768, 64]
    return full[None].astype(np.float32)

